# revision 14
# baseline (speedup 1.0000x reference)
"""Bass/Tile kernel for nn_BinaryClassifierChain on 8 trn2 cores.

Math (per reference.py):
  wc   = softmax(word_class_features, axis=0)            # over batch dim
  base = concat([features, wc], -1)                      # [B, W, 1088]
  L    = base @ W[:, :1088].T + b                        # [B, W, 32]
  chain: p_i = sigmoid(L_i + sum_{j<i} Wbin[i, j] p_j)   # Wbin = W[:, 1088:]

Sharding: data-parallel over the words dim (1024 = 8 x 128); softmax
couples the batch dim, which stays whole per shard.

v3 dataflow per core (vs v2 baseline at ~295us):
  - all 16 feature cast-DMAs queued at t=0 (SWDGE, bufs=4)
  - softmax on ACT/DVE; wc transposed on-chip by PE (64 bf16 [128,64]
    transposes) into wct3 [c, b, w] -- no DRAM roundtrip
  - wc contribution folded into the corner-turn psum accumulation:
    per batch, transpose(blt) (start) then matmul(wct3[:,b,:], wcW)
    (accumulate) -> ptc; ACT evacuates ptc -> L.  Emission is lagged
    (burst B0..B4 after A4) so PE never stalls on wct3.
  - chain: batches 0-31 on DVE interleaved with groups 8-15 of the
    main loop; post-loop tail = q2 (DVE) || q3 (gpsimd muls + DVE
    reduces); corr is bf16 to enable the DVE 2x all-16-bit mode; mul
    widths rounded to even (P slots >= i are zero, Wbin row is dense)
  - store: ACT casts P quarters to f32, HWDGE stores (no SWDGE
    descriptor-generation tail)
"""

import sys

sys.path.insert(0, "/opt/trn_rl_repo")

import numpy as np
import orjson

import concourse.bass as bass
import concourse.mybir as mybir
import concourse.tile as tile
from concourse import masks
from concourse.bass_utils import run_bass_kernel_spmd

F32 = mybir.dt.float32
BF16 = mybir.dt.bfloat16
AF = mybir.ActivationFunctionType
ALU = mybir.AluOpType
AX = mybir.AxisListType

B = 64          # batch
NWALL = 1024    # total words
NCORES = 8
NW = NWALL // NCORES  # 128 words per core
D = 1024        # embed dim
C = 64          # word classes
NB = 32         # bin features
DIN = D + C + NB  # 1120
GRP = 4         # batches per matmul group (4 * 128 words = 512 tokens)
NGRP = B // GRP


def _split_multiwait_json(raw: bytes) -> bytes:
    """walrus in this container only accepts 1 sync-wait per most
    instructions; Tile's final drain (and some others) carry several.
    Move extras onto preceding EventSemaphore carriers (2 waits each) on
    the same engine."""
    bir = orjson.loads(raw)
    for fn in bir["functions"]:
        for blk in fn["blocks"]:
            out = []
            for ins in blk["instructions"]:
                si = ins.get("sync_info")
                waits = (si or {}).get("on_wait") or []
                if len(waits) > 1:
                    extra = waits[:-1]
                    for k in range(0, len(extra), 2):
                        out.append(
                            {
                                "debug": ins.get("debug", 0),
                                "engine": ins["engine"],
                                "ins": [],
                                "outs": [],
                                "name": f"{ins['name']}_sw{k}",
                                "opcode": "EventSemaphore",
                                "sync_info": {
                                    "on_update": [],
                                    "on_wait": extra[k : k + 2],
                                },
                            }
                        )
                    si["on_wait"] = [waits[-1]]
                out.append(ins)
            blk["instructions"] = out
    return orjson.dumps(bir)


def _even(i: int) -> int:
    return i + (i & 1)


def build_program():
    nc = bass.Bass("TRN2", target_bir_lowering=False, debug=False)

    feat = nc.dram_tensor("feat", [B, NW, D], F32, kind="ExternalInput")
    wc = nc.dram_tensor("wc", [B, NW, C], F32, kind="ExternalInput")
    Wt = nc.dram_tensor("W", [NB, DIN], F32, kind="ExternalInput")
    bt = nc.dram_tensor("b", [NB], F32, kind="ExternalInput")
    out = nc.dram_tensor("out", [B, NW, NB], F32, kind="ExternalOutput")

    with tile.TileContext(nc) as tc:
        with (
            tc.tile_pool(name="const", bufs=1) as constp,
            tc.tile_pool(name="x2", bufs=4) as x2p,
            tc.tile_pool(name="xt", bufs=2) as xtp,
            tc.tile_pool(name="blt", bufs=7) as bltp,
            tc.tile_pool(name="lp", bufs=1) as lpp,
            tc.tile_pool(name="pf", bufs=2) as pfp,
            tc.tile_pool(name="tp", bufs=3, space="PSUM") as tpp,
            tc.tile_pool(name="mmps", bufs=2, space="PSUM") as mmpsp,
            tc.tile_pool(name="petps", bufs=1, space="PSUM") as petpsp,
        ):
            # ---------------- constants / weights ----------------
            ident = constp.tile([128, 128], BF16)
            masks.make_identity(nc, ident[:])
            identf = constp.tile([NB, NB], F32)
            masks.make_identity(nc, identf[:])

            b_sb = constp.tile([NB, 1], F32)
            nc.sync.dma_start(b_sb[:], bt.ap().unsqueeze(1))

            # W cast to bf16, padded to 1152 cols so 128-col xbar chunks
            # cover it; chunk 8 rows 0:64 are the wc columns.  Only the
            # 32-col pad needs the memset.
            wbf = constp.tile([NB, 1152], BF16)
            nc.gpsimd.memset(wbf[:, DIN:1152], 0.0)
            nc.gpsimd.dma_start(wbf[:, 0:DIN], Wt.ap())
            # Wbin rows (bf16) for the chain broadcast, loaded before the
            # big feature loads so the Q7 queue reaches them instantly
            wbin1b = constp.tile([1, NB * NB], BF16)
            nc.gpsimd.dma_start(wbin1b[:], Wt.ap()[:, D + C : DIN].unsqueeze(0))
            ones1 = constp.tile([1, 128], BF16)
            nc.vector.memset(ones1[:], 1.0)
            wrepb = constp.tile([128, NB * NB], BF16)

            wtr = constp.tile([128, 9, NB], BF16)
            for k in range(9):
                nc.sync.dma_start(
                    wtr[:, k, :], wbf[:, k * 128 : (k + 1) * 128], transpose=True
                )
            # block-diag wc weights [ (2b,64c), (2b,32bin) ] for the pair
            # matmuls in body_B
            wcd2 = constp.tile([128, 64], BF16)
            nc.vector.memset(wcd2[:], 0.0)
            nc.vector.tensor_copy(wcd2[0:C, 0:NB], wtr[0:C, 8, :])
            nc.sync.dma_start(wcd2[C:128, NB:64], wtr[0:C, 8, :])

            # softmaxed wc: token-major wcn, and batch-pair transposed
            # wctP [(2b,64c), pair, w] built by 32 sync DMA transposes
            wctP = constp.tile([128, B // 2, NW], BF16)
            wcn = constp.tile([128, B, C], BF16)

            # chain state
            L = lpp.tile([128, B, NB], F32)
            P = lpp.tile([128, B, NB], BF16)
            tmp0 = lpp.tile([128, 32, NB], BF16)
            tmp2 = lpp.tile([128, 16, NB], BF16)
            tmpg = lpp.tile([128, 16, NB], BF16)
            cor0 = lpp.tile([128, 32], BF16)
            cor2 = lpp.tile([128, 16], BF16)
            cor3 = lpp.tile([128, 16], BF16)
            nc.vector.memset(P[:], 0.0)

            # ---------------- softmax over batch ----------------
            with tc.tile_pool(name="soft", bufs=1) as softp:
                # load via SWDGE so it lands AHEAD of the feature loads in
                # the Q7 queue (a lone HWDGE load starves in the SDMA
                # round-robin against 16 SWDGE feature streams)
                wcs = softp.tile([128, B, C], F32)
                nc.gpsimd.dma_start(wcs[:], wc.ap().rearrange("b p c -> p b c"))
                ex = softp.tile([128, B, C], F32)
                nc.scalar.activation(ex[:], wcs[:], AF.Exp)
                acc = softp.tile([128, B // 2, C], F32)
                nc.vector.tensor_add(
                    acc[:], ex[:, 0 : B // 2, :], ex[:, B // 2 : B, :]
                )
                h = B // 4
                while h >= 1:
                    nc.vector.tensor_add(
                        acc[:, 0:h, :], acc[:, 0:h, :], acc[:, h : 2 * h, :]
                    )
                    h //= 2
                rec = softp.tile([128, C], F32)
                nc.vector.reciprocal(rec[:], acc[:, 0, :])
                nc.vector.tensor_tensor(
                    wcn[:],
                    ex[:],
                    rec[:].unsqueeze(1).broadcast_to([128, B, C]),
                    op=ALU.mult,
                )
                # transpose batch-pairs [128w, 2b*64c] -> [(2b,64c), 128w]
                # on the otherwise-idle sync engine
                for p in range(B // 2):
                    nc.sync.dma_start(
                        wctP[:, p, :], wcn[:, 2 * p : 2 * p + 2, :], transpose=True
                    )

            # ---------------- helpers ----------------
            x2_tiles = []

            def emit_load(g):
                b0 = g * GRP
                x2 = x2p.tile([128, GRP, D], BF16, tag="x2")
                nc.gpsimd.dma_start(
                    x2[:], feat.ap()[b0 : b0 + GRP, :, :].rearrange("b p d -> p b d")
                )
                x2_tiles.append(x2)

            blts = {}

            def body_A(g):
                x2 = x2_tiles[g]
                xts = xtp.tile([128, 8, GRP * 128], BF16, tag="xt")
                evac = nc.vector if g % 2 == 0 else nc.scalar
                for kh in range(4):
                    pt = tpp.tile([128, 2, GRP * 128], BF16, tag="xtps")
                    for kk in range(2):
                        k = kh * 2 + kk
                        for bi in range(GRP):
                            nc.tensor.transpose(
                                pt[:, kk, bi * 128 : (bi + 1) * 128],
                                x2[:, bi, k * 128 : (k + 1) * 128],
                                ident[:],
                            )
                    if g % 2 == 0:
                        nc.vector.tensor_copy(xts[:, kh * 2 : kh * 2 + 2, :], pt[:])
                    else:
                        nc.scalar.copy(xts[:, kh * 2 : kh * 2 + 2, :], pt[:])
                ps = mmpsp.tile([NB, 512], F32, tag="mm")
                for k in range(8):
                    nc.tensor.matmul(
                        ps[:], wtr[:, k, :], xts[:, k, :],
                        start=(k == 0), stop=(k == 7),
                    )
                blt = bltp.tile([NB, 512], F32, tag="blt")
                nc.scalar.activation(
                    blt[:], ps[:], AF.Identity, bias=b_sb[:, 0:1], scale=1.0
                )
                blts[g] = blt

            def body_B(g):
                # corner turn into ptc, wc contribution via batch-pair
                # matmuls into wcp (wctP pair stationary, block-diag wcd2
                # moving); DVE sums both psums into L
                blt = blts[g]
                ptc = petpsp.tile([128, GRP, NB], F32, tag="pet")
                for q in range(GRP):
                    nc.tensor.transpose(
                        ptc[:, q, :], blt[:, q * 128 : (q + 1) * 128], identf[:]
                    )
                wcp = petpsp.tile([128, GRP, NB], F32, tag="wcp")
                for h in range(2):
                    nc.tensor.matmul(
                        wcp[:, 2 * h : 2 * h + 2, :], wctP[:, 2 * g + h, :], wcd2[:],
                        start=True, stop=True,
                    )
                wcsb = bltp.tile([128, GRP, NB], F32, tag="wcsb")
                nc.scalar.copy(wcsb[:], wcp[:])
                b0 = g * GRP
                nc.vector.scalar_tensor_tensor(
                    L[:, b0 : b0 + GRP, :], ptc[:], 1.0, wcsb[:],
                    op0=ALU.mult, op1=ALU.add,
                )

            def chain_step(i, bs, tmp, cor, mul_eng, stt_eng):
                # one chain bin for batches bs (a slice); P[:, :, j>=i] == 0
                nb_ = bs.stop - bs.start
                if i > 0:
                    ie = _even(i)
                    wrow = wrepb[:, i * NB : i * NB + ie]
                    mul_eng.tensor_tensor(
                        tmp[:, 0:nb_, 0:ie],
                        P[:, bs, 0:ie],
                        wrow.unsqueeze(1).broadcast_to([128, nb_, ie]),
                        op=ALU.mult,
                    )
                    nc.vector.tensor_reduce(
                        cor[:, 0:nb_], tmp[:, 0:nb_, 0:ie], axis=AX.X, op=ALU.add
                    )
                    stt_eng.scalar_tensor_tensor(
                        L[:, bs, i], cor[:, 0:nb_], 1.0, L[:, bs, i],
                        op0=ALU.mult, op1=ALU.add,
                    )
                nc.scalar.activation(P[:, bs, i], L[:, bs, i], AF.Sigmoid)

            def emit_store(bq):
                # cast 16 batches to f32 on ACT, store via HWDGE
                pf = pfp.tile([128, 16, NB], F32, tag="pf")
                nc.scalar.copy(pf[:], P[:, bq : bq + 16, :])
                nc.sync.dma_start(
                    out.ap()[bq : bq + 16, :, :].rearrange("b p i -> p b i"), pf[:]
                )

            # ---------------- emission schedule ----------------
            for g in range(NGRP):
                emit_load(g)

            with nc.allow_low_precision(reason="bf16 chain corr, products ~1e-1"):
                for g in range(NGRP):
                    body_A(g)
                    if g == 0:
                        # Wbin broadcast to all partitions via k=1 matmuls
                        for h in range(2):
                            wps = tpp.tile([128, 512], F32, tag="xtps")
                            nc.tensor.matmul(
                                wps[:], ones1[:],
                                wbin1b[:, h * 512 : (h + 1) * 512],
                                start=True, stop=True,
                            )
                            nc.vector.tensor_copy(
                                wrepb[:, h * 512 : (h + 1) * 512], wps[:]
                            )
                    if g == 2:
                        body_B(0)
                        body_B(1)
                    if g >= 3:
                        body_B(g - 1)
                    if g == 15:
                        body_B(15)
                    if g >= 8:
                        # chain h0 (batches 0-31) on DVE, 4 bins per group
                        for i in range(4 * (g - 8), 4 * (g - 8) + 4):
                            chain_step(
                                i, slice(0, 32), tmp0, cor0, nc.vector, nc.vector
                            )

                emit_store(0)
                emit_store(16)

                # tail: q2 (b32-47) on DVE || q3 (b48-63) gpsimd muls
                for i in range(NB):
                    bs2, bs3 = slice(32, 48), slice(48, 64)
                    if i > 0:
                        ie = _even(i)
                        wrow = wrepb[:, i * NB : i * NB + ie]
                        nc.vector.tensor_tensor(
                            tmp2[:, :, 0:ie], P[:, bs2, 0:ie],
                            wrow.unsqueeze(1).broadcast_to([128, 16, ie]),
                            op=ALU.mult,
                        )
                        nc.gpsimd.tensor_tensor(
                            tmpg[:, :, 0:ie], P[:, bs3, 0:ie],
                            wrow.unsqueeze(1).broadcast_to([128, 16, ie]),
                            op=ALU.mult,
                        )
                        nc.vector.tensor_reduce(
                            cor2[:], tmp2[:, :, 0:ie], axis=AX.X, op=ALU.add
                        )
                        nc.vector.scalar_tensor_tensor(
                            L[:, bs2, i], cor2[:], 1.0, L[:, bs2, i],
                            op0=ALU.mult, op1=ALU.add,
                        )
                        nc.vector.tensor_reduce(
                            cor3[:], tmpg[:, :, 0:ie], axis=AX.X, op=ALU.add
                        )
                        nc.gpsimd.tensor_tensor(
                            L[:, bs3, i], cor3[:], L[:, bs3, i], op=ALU.add
                        )
                    nc.scalar.activation(P[:, bs2, i], L[:, bs2, i], AF.Sigmoid)
                    nc.scalar.activation(P[:, bs3, i], L[:, bs3, i], AF.Sigmoid)

                emit_store(32)
                emit_store(48)

    orig = nc.to_json_bytes
    nc.to_json_bytes = lambda: _split_multiwait_json(orig())
    return nc


_PROG = None


def _get_prog():
    global _PROG
    if _PROG is None:
        _PROG = build_program()
    return _PROG


def kernel(features, word_class_features, W, b, trace=False, tmpdir=None):
    features = np.ascontiguousarray(features, dtype=np.float32)
    word_class_features = np.ascontiguousarray(word_class_features, dtype=np.float32)
    W = np.ascontiguousarray(W, dtype=np.float32)
    b = np.ascontiguousarray(b, dtype=np.float32)

    nc = _get_prog()
    in_maps = []
    for c in range(NCORES):
        sl = slice(c * NW, (c + 1) * NW)
        in_maps.append(
            {
                "feat": np.ascontiguousarray(features[:, sl, :]),
                "wc": np.ascontiguousarray(word_class_features[:, sl, :]),
                "W": W,
                "b": b,
            }
        )
    res = run_bass_kernel_spmd(
        nc, in_maps, core_ids=list(range(NCORES)), trace=trace, tmpdir=tmpdir
    )
    outp = np.concatenate([res.results[c]["out"] for c in range(NCORES)], axis=1)
    kernel._last_result = res
    return outp
